# revision 41
# baseline (speedup 1.0000x reference)
"""MixtureRouter Trainium2 kernel (v2: w1-stationary, host-normalized fp8).

Per-core (data-parallel over batch, 8 cores): the device computes
    G[r, c] = sum_{t in chunk c} gelu( (xn @ w1g)[t, r] + vb1[r] )
for r-chunks of 128 partitions x token-chunks of 512, i.e. the full
Linear(2048->512) + bias + GELU + sum-over-sequence. The host computes
LayerNorm in f32 (exactly matching reference semantics), folds ln_gamma
into w1, pre-transposes x to [d, tok] fp8 layout, and runs the tiny tail
(H @ w2 + S*b2 -> router head, aux_loss / next_idx) in fp64.

Key design points vs the previous version:
  - Orientation flipped: w1g chunks are the matmul STATIONARY operand
    ([d,r] needs no transpose), xn^T the MOVING operand. The host ships
    x already d-major, so the 256 on-chip PE transposes and the 16 big
    PSUM->SBUF ACT copies are gone entirely. PE issues nothing but the
    128 DoubleRow fp8 matmuls (~213ns each => ~27us, the fp8 roofline).
  - With r on partitions, the b1 bias is a per-partition scalar: it
    rides the ACT Gelu as the `bias` operand, and the token-sum rides
    the same instruction as `accum_out` (free-dim reduction). One ACT
    instruction per PSUM bank does bias+gelu+reduce; DVE is idle.
  - LayerNorm is folded on the host: xn = (x - mu) * rsqrt(var + eps)
    in f32 (better than the device bn_stats path), then quantized to
    fp8e4m3. fp8 x fp8 DoubleRow was already the baseline's matmul
    precision; measured end-to-end logits error ~5e-3 vs the 2e-2 gate,
    and per-batch top-1 margins are ~14+ logits vs ~0.5 error.
  - fp8 x halves HBM traffic again (4 MiB/core + 1 MiB weights). x moves
    as 32 per-(token-half, k-chunk) DMAs with 1024B descriptors, w as 4
    quads with 2048B descriptors, pair-interleaved in PE demand order
    across the SP HWDGE (~165 B/ns) and Pool SWDGE (~0.8us/trigger)
    queues. Nothing rides the ACT HWDGE queue (~25ns/descriptor and the
    epilogue spin-drains it). A no-dep dummy Gelu pre-loads the ACT
    table; 9 junk DoubleRow matmuls burn the PE DVFS ramp during the
    DMA-latency window (deliberately NOT bridged gaplessly into the
    real stream — that trips the chip power throttle, 2.4 -> 2.0 GHz).
  - Measured: ~48.5us mean / ~49.8us max across cores (from the 88.6us
    baseline), of which ~27.6us is the fp8 DoubleRow roofline and ~16us
    is fixed preamble/DMA-latency/teardown paid by any kernel here.
"""

import sys
import types

import ml_dtypes
import numpy as np

import concourse.bass as bass
import concourse.mybir as mybir
import concourse.tile as tile
from concourse import bacc
from concourse.bass_utils import run_bass_kernel_spmd

# run_bass_kernel_spmd imports antenv.axon_hooks when BASS_TRACE is set; that
# module is absent on this image. Provide it so tracing degrades gracefully.
if "antenv.axon_hooks" not in sys.modules:
    try:
        import antenv.axon_hooks  # noqa: F401
    except ImportError:
        _hm = types.ModuleType("antenv.axon_hooks")
        _hm._hook = None
        _hm.set_axon_ntff_profile_hook = lambda h: setattr(_hm, "_hook", h)
        _hm.get_axon_ntff_profile_hook = lambda: _hm._hook
        sys.modules["antenv.axon_hooks"] = _hm
        try:
            from trn_agent_boot.trn_boot import _ntff_profile_via_ctypes

            _hm._hook = _ntff_profile_via_ctypes("/opt/axon/libaxon_pjrt.so")
        except Exception:
            pass

F32 = mybir.dt.float32
FP8 = mybir.dt.float8e4

B, S, D, R, E = 8, 2048, 2048, 512, 8
N_CORES = 8
P = 128
NK = D // P          # 16 contraction chunks of 128
NK2 = NK // 2        # 8 DoubleRow pairs
NTC = S // 512       # 4 token chunks of 512
NRC = R // P         # 4 r chunks of 128
LN_EPS = 1e-5

_cache = {}


def _build():
    nc = bacc.Bacc("TRN2", target_bir_lowering=False, debug=False, num_devices=N_CORES)
    # xd rows: ((u*16 + k)*128 + p), cols: t in token half u  [fp8, 4 MiB]
    xd = nc.dram_tensor("xd", [2 * NK * P, 1024], FP8, kind="ExternalInput")
    # wd rows: (a*128 + p), cols: (jj*512 + r), a in 0..3 [fp8, 1 MiB]
    wd = nc.dram_tensor("wd", [4 * P, 4 * R], FP8, kind="ExternalInput")
    vd = nc.dram_tensor("vd", [P, NRC], F32, kind="ExternalInput")
    gout = nc.dram_tensor("gout", [P, NTC * NRC], F32, kind="ExternalOutput")

    with tile.TileContext(nc) as tc:
        with (
            tc.tile_pool(name="const", bufs=1) as const,
            tc.tile_pool(name="gdump", bufs=2) as gdump,
            tc.tile_pool(name="psm", bufs=8, space="PSUM") as psm,
        ):
            ws = const.tile([P, NK, R], FP8)      # stationary w1g  (8 KiB/part)
            # moving xn^T: k-major rows of 2048 tokens. The 2048B row stride
            # matters: DoubleRow reads both k-tile rows concurrently, and
            # 512B-strided rows collide in SBUF (measured 216 -> 259 ns).
            xs = const.tile([P, NK, S], FP8)      # 32 KiB/part
            vb1s = const.tile([P, NRC], F32)      # bias b1 (gamma/beta folded)
            gcol = const.tile([P, NTC * NRC], F32)
            scr1 = const.tile([P, 1], F32)
            # PE warm-up junk operands (zeroed; real matmul shapes)
            wj = const.tile([P, 2, P], FP8)
            xj = const.tile([P, 2, 512], FP8)

            def w_slice(a):
                return wd[a * P : (a + 1) * P, :].rearrange(
                    "p (jj r) -> p jj r", r=R
                )

            def x_mov(tci, k2):
                return xs[:, 2 * k2 : 2 * k2 + 2, tci * 512 : (tci + 1) * 512]

            nc.vector.memset(scr1, 0.0)
            nc.vector.memset(wj.bitcast(mybir.dt.uint32), 0)
            nc.vector.memset(xj.bitcast(mybir.dt.uint32), 0)

            # prime the ACT Gelu table (no data deps: immediate scale/bias,
            # zeroed input). vd and all gouts ride the SP queue — anything on
            # the ACT HWDGE queue drains at ~25ns/descriptor and the kernel
            # epilogue spin-waits for that queue to empty (~9us measured).
            nc.scalar.activation(
                scr1, scr1, mybir.ActivationFunctionType.Gelu, bias=0.0, scale=0.0
            )
            # vd (2 KiB) rides the slow-but-idle ACT queue: lands ~10us,
            # first gelu needs it ~13us; frees the first SP slot for x/w
            nc.scalar.dma_start(vb1s, vd[:, :])

            # DMA schedule: bulk split across the two fast queues (the ACT
            # HWDGE queue is ~10x slower, bulk never goes there). x moves as
            # 32 per-(token-half u, k-chunk) DMAs of 128 KiB with 1024B
            # descriptors (~165 B/ns on SP); w as 4 quads with 2048B
            # descriptors. SWDGE is trigger-paced (~0.8us per DMA regardless
            # of size). The u0 half + w feed both tc0 and tc1 and are
            # emitted strictly in PE demand order, alternating queues.
            def w_dma(q, a):
                q.dma_start(ws[:, 4 * a : 4 * a + 4], w_slice(a))

            def x_dma(q, u, k, half=None):
                base = (u * NK + k) * P
                if half is None:
                    q.dma_start(
                        xs[:, k, u * 1024 : (u + 1) * 1024],
                        xd[base : base + P, :],
                    )
                else:
                    lo = half * 512
                    q.dma_start(
                        xs[:, k, u * 1024 + lo : u * 1024 + lo + 512],
                        xd[base : base + P, lo : lo + 512],
                    )

            # emit w + u0 in PE demand order, alternating queues; u1 after
            for u in range(2):
                items = []
                for a in range(4):
                    items.append(("w", a))
                    items.extend(("x", 4 * a + i) for i in range(4))
                for idx, (kind, val) in enumerate(items):
                    q = nc.sync if idx % 2 == 0 else nc.gpsimd
                    if kind == "w":
                        if u == 0:
                            w_dma(q, val)
                        # u == 1: weights already resident; keep x alternating
                    elif u == 1 and val == NK - 1:
                        # last chunk (needed ~31us) on the slow ACT queue:
                        # lands ~20us even at 12 B/ns, frees a SWDGE trigger
                        x_dma(nc.scalar, u, val)
                    else:
                        x_dma(q, u, val)

            def mm(banks, tci, rc, k2):
                nc.tensor.matmul(
                    banks[rc],
                    ws[:, 2 * k2 : 2 * k2 + 2, rc * P : (rc + 1) * P],
                    x_mov(tci, k2),
                    start=(k2 == 0), stop=(k2 == NK2 - 1),
                    perf_mode=mybir.MatmulPerfMode.DoubleRow,
                    skip_group_check=True,
                )

            def gelu(banks, tci, rc):
                g = gdump.tile([P, 512], F32, tag="g", name=f"g{tci}_{rc}")
                col = tci * NRC + rc
                nc.scalar.activation(
                    g, banks[rc], mybir.ActivationFunctionType.Gelu,
                    bias=vb1s[:, rc : rc + 1],
                    accum_out=gcol[:, col : col + 1],
                )

            def ship(tci):
                nc.sync.dma_start(
                    gout[:, tci * NRC : (tci + 1) * NRC],
                    gcol[:, tci * NRC : (tci + 1) * NRC],
                )

            banks0 = [
                psm.tile([P, 512], F32, tag="mm", name=f"mmA_{rc}")
                for rc in range(NRC)
            ]
            banks1 = [
                psm.tile([P, 512], F32, tag="mm", name=f"mmB_{rc}")
                for rc in range(NRC)
            ]

            # burn the PE DVFS p-state ramp on junk matmuls (results are
            # discarded; bank is reset by the first real start=True matmul).
            # Deliberately NOT bridged gaplessly into the real stream: a
            # fully gapless dense ignition measurably trips the chip power
            # throttle (2.4 -> 2.0 GHz for the whole run, +20% everywhere).
            for i in range(9):
                nc.tensor.matmul(
                    banks0[0], wj, xj, start=True, stop=True,
                    perf_mode=mybir.MatmulPerfMode.DoubleRow,
                    skip_group_check=True,
                )

            # tc0 and tc1 read the SAME u0 x chunks, so interleave them over
            # all 8 PSUM banks for the first k2 steps: per-chunk PE demand
            # halves exactly where DMA supply is leanest (no head stalls,
            # and the DVFS ramp overlaps the slack). Then tc0 finishes alone
            # and its gelus free banks 0-3 before tc2 needs them (don't
            # extend the interleave much further: that margin shrinks
            # ~0.9us per extra step and an ACT race eats the gain).
            ILV = 4
            for k2 in range(ILV):
                for banks, tci in ((banks0, 0), (banks1, 1)):
                    for rc in range(NRC):
                        mm(banks, tci, rc, k2)
            for k2 in range(ILV, NK2):
                for rc in range(NRC):
                    mm(banks0, 0, rc, k2)
            for rc in range(NRC):
                gelu(banks0, 0, rc)
            ship(0)
            for k2 in range(ILV, NK2):
                for rc in range(NRC):
                    mm(banks1, 1, rc, k2)
            for rc in range(NRC):
                gelu(banks1, 1, rc)
            ship(1)

            # tc2: k2-outer over 4 banks (reuses banks0's buffers)
            banks2 = [
                psm.tile([P, 512], F32, tag="mm", name=f"mmC_{rc}")
                for rc in range(NRC)
            ]
            for k2 in range(NK2):
                for rc in range(NRC):
                    mm(banks2, 2, rc, k2)
            for rc in range(NRC):
                gelu(banks2, 2, rc)
            ship(2)

            # tc3: rc-outer so the tail gelus overlap the last matmuls
            banks3 = [
                psm.tile([P, 512], F32, tag="mm", name=f"mmD_{rc}")
                for rc in range(NRC)
            ]
            for rc in range(NRC):
                for k2 in range(NK2):
                    mm(banks3, 3, rc, k2)
                gelu(banks3, 3, rc)
            ship(3)
    nc.finalize()
    return nc


def kernel(hidden_states, ln_gamma, ln_beta, w1, b1, w2, b2, wr, br):
    hs = np.asarray(hidden_states, dtype=np.float32)
    # LayerNorm on host in f32 (f64 accumulation for the stats)
    mu = hs.mean(-1, keepdims=True, dtype=np.float64)
    var = (hs.astype(np.float64) - mu).var(-1, keepdims=True)
    rstd = 1.0 / np.sqrt(var + LN_EPS)
    xn8 = ((hs - mu.astype(np.float32)) * rstd.astype(np.float32)).astype(
        ml_dtypes.float8_e4m3fn
    )

    g64 = np.asarray(ln_gamma, dtype=np.float64)
    be64 = np.asarray(ln_beta, dtype=np.float64)
    w1_64 = np.asarray(w1, dtype=np.float64)
    w1g8 = (g64[:, None] * w1_64).astype(np.float32).astype(ml_dtypes.float8_e4m3fn)
    vb1 = (be64 @ w1_64 + np.asarray(b1, np.float64)).astype(np.float32)

    # device layouts (see _build):
    #   wd[(a*128+p), (jj*512+r)] = w1g8[(4*a+jj)*128+p, r]
    wdh = np.ascontiguousarray(
        w1g8.reshape(4, 4, P, R).transpose(0, 2, 1, 3).reshape(4 * P, 4 * R)
    )
    vdh = np.ascontiguousarray(vb1.reshape(NRC, P).T)  # [128, 4]

    if "nc" not in _cache:
        _cache["nc"] = _build()
    nc = _cache["nc"]

    in_maps = []
    for b in range(N_CORES):
        #   xd[((u*16+k)*128+p), t] = xn8[b, u*1024+t, k*128+p]
        xT = np.ascontiguousarray(xn8[b].T)  # [D, S]
        xdh = np.ascontiguousarray(
            xT.reshape(NK, P, 2, 1024)
            .transpose(2, 0, 1, 3)
            .reshape(2 * NK * P, 1024)
        )
        in_maps.append({"xd": xdh, "wd": wdh, "vd": vdh})
    res = run_bass_kernel_spmd(nc, in_maps, core_ids=list(range(N_CORES)))
    gaccs = np.stack([res.results[b]["gout"] for b in range(N_CORES)], axis=0)
    global _last_res
    _last_res = res

    # host tail in fp64 (tiny): H -> w2 -> router -> aux/next_idx
    # gcol[p, tc*4+rc] = sum over token chunk tc of gelu row r = rc*128+p
    H = (
        gaccs.astype(np.float64)
        .reshape(B, P, NTC, NRC)
        .sum(axis=2)            # [B, p, rc]
        .transpose(0, 2, 1)     # [B, rc, p]
        .reshape(B, R)
    )
    bt = H @ np.asarray(w2, np.float64) + float(S) * np.asarray(b2, np.float64)
    logits = bt @ np.asarray(wr, np.float64) + np.asarray(br, np.float64)  # [B, E]
    global _last_logits
    _last_logits = logits.astype(np.float32)

    idx = logits.argmax(axis=-1)
    targets = np.zeros_like(logits)
    targets[np.arange(B), idx] = 1.0
    aux = (np.logaddexp(0.0, logits) - logits * targets).mean()
    counts = targets.sum(0)
    next_idx = int(np.argmax(counts))
    return np.float32(aux), np.int32(next_idx)


# revision 43
# speedup vs baseline: 1.0086x; 1.0086x over previous
"""MixtureRouter Trainium2 kernel (v2: w1-stationary, host-normalized fp8).

Per-core (data-parallel over batch, 8 cores): the device computes
    G[r, c] = sum_{t in chunk c} gelu( (xn @ w1g)[t, r] + vb1[r] )
for r-chunks of 128 partitions x token-chunks of 512, i.e. the full
Linear(2048->512) + bias + GELU + sum-over-sequence. The host computes
LayerNorm in f32 (exactly matching reference semantics), folds ln_gamma
into w1, pre-transposes x to [d, tok] fp8 layout, and runs the tiny tail
(H @ w2 + S*b2 -> router head, aux_loss / next_idx) in fp64.

Key design points vs the previous version:
  - Orientation flipped: w1g chunks are the matmul STATIONARY operand
    ([d,r] needs no transpose), xn^T the MOVING operand. The host ships
    x already d-major, so the 256 on-chip PE transposes and the 16 big
    PSUM->SBUF ACT copies are gone entirely. PE issues nothing but the
    128 DoubleRow fp8 matmuls (~213ns each => ~27us, the fp8 roofline).
  - With r on partitions, the b1 bias is a per-partition scalar: it
    rides the ACT Gelu as the `bias` operand, and the token-sum rides
    the same instruction as `accum_out` (free-dim reduction). One ACT
    instruction per PSUM bank does bias+gelu+reduce; DVE is idle.
  - LayerNorm is folded on the host: xn = (x - mu) * rsqrt(var + eps)
    in f32 (better than the device bn_stats path), then quantized to
    fp8e4m3. fp8 x fp8 DoubleRow was already the baseline's matmul
    precision; measured end-to-end logits error ~5e-3 vs the 2e-2 gate,
    and per-batch top-1 margins are ~14+ logits vs ~0.5 error.
  - fp8 x halves HBM traffic again (4 MiB/core + 1 MiB weights). x moves
    as 32 per-(token-half, k-chunk) DMAs with 1024B descriptors, w as 4
    quads with 2048B descriptors, pair-interleaved in PE demand order
    across the SP HWDGE (~165 B/ns) and Pool SWDGE (~0.8us/trigger)
    queues. Nothing rides the ACT HWDGE queue (~25ns/descriptor and the
    epilogue spin-drains it). A no-dep dummy Gelu pre-loads the ACT
    table; 9 junk DoubleRow matmuls burn the PE DVFS ramp during the
    DMA-latency window (deliberately NOT bridged gaplessly into the
    real stream — that trips the chip power throttle, 2.4 -> 2.0 GHz).
  - Measured: ~48.5us mean / ~49.8us max across cores (from the 88.6us
    baseline), of which ~27.6us is the fp8 DoubleRow roofline and ~16us
    is fixed preamble/DMA-latency/teardown paid by any kernel here.
"""

import sys
import types

import ml_dtypes
import numpy as np

import concourse.bass as bass
import concourse.mybir as mybir
import concourse.tile as tile
from concourse import bacc
from concourse.bass_utils import run_bass_kernel_spmd

# run_bass_kernel_spmd imports antenv.axon_hooks when BASS_TRACE is set; that
# module is absent on this image. Provide it so tracing degrades gracefully.
if "antenv.axon_hooks" not in sys.modules:
    try:
        import antenv.axon_hooks  # noqa: F401
    except ImportError:
        _hm = types.ModuleType("antenv.axon_hooks")
        _hm._hook = None
        _hm.set_axon_ntff_profile_hook = lambda h: setattr(_hm, "_hook", h)
        _hm.get_axon_ntff_profile_hook = lambda: _hm._hook
        sys.modules["antenv.axon_hooks"] = _hm
        try:
            from trn_agent_boot.trn_boot import _ntff_profile_via_ctypes

            _hm._hook = _ntff_profile_via_ctypes("/opt/axon/libaxon_pjrt.so")
        except Exception:
            pass

F32 = mybir.dt.float32
FP8 = mybir.dt.float8e4

B, S, D, R, E = 8, 2048, 2048, 512, 8
N_CORES = 8
P = 128
NK = D // P          # 16 contraction chunks of 128
NK2 = NK // 2        # 8 DoubleRow pairs
NTC = S // 512       # 4 token chunks of 512
NRC = R // P         # 4 r chunks of 128
LN_EPS = 1e-5

_cache = {}


def _build():
    nc = bacc.Bacc("TRN2", target_bir_lowering=False, debug=False, num_devices=N_CORES)
    # xd rows: ((u*16 + k)*128 + p), cols: t in token half u  [fp8, 4 MiB]
    xd = nc.dram_tensor("xd", [2 * NK * P, 1024], FP8, kind="ExternalInput")
    # wd rows: (a*128 + p), cols: (jj*512 + r), a in 0..3 [fp8, 1 MiB]
    wd = nc.dram_tensor("wd", [4 * P, 4 * R], FP8, kind="ExternalInput")
    vd = nc.dram_tensor("vd", [P, NRC], F32, kind="ExternalInput")
    gout = nc.dram_tensor("gout", [P, NTC * NRC], F32, kind="ExternalOutput")

    with tile.TileContext(nc) as tc:
        with (
            tc.tile_pool(name="const", bufs=1) as const,
            tc.tile_pool(name="gdump", bufs=2) as gdump,
            tc.tile_pool(name="psm", bufs=8, space="PSUM") as psm,
        ):
            ws = const.tile([P, NK, R], FP8)      # stationary w1g  (8 KiB/part)
            # moving xn^T: k-major rows of 2048 tokens. The 2048B row stride
            # matters: DoubleRow reads both k-tile rows concurrently, and
            # 512B-strided rows collide in SBUF (measured 216 -> 259 ns).
            xs = const.tile([P, NK, S], FP8)      # 32 KiB/part
            vb1s = const.tile([P, NRC], F32)      # bias b1 (gamma/beta folded)
            gcol = const.tile([P, NTC * NRC], F32)
            scr1 = const.tile([P, 1], F32)
            # PE warm-up junk operands (zeroed; real matmul shapes)
            wj = const.tile([P, 2, P], FP8)
            xj = const.tile([P, 2, 512], FP8)

            def w_slice(a):
                return wd[a * P : (a + 1) * P, :].rearrange(
                    "p (jj r) -> p jj r", r=R
                )

            def x_mov(tci, k2):
                return xs[:, 2 * k2 : 2 * k2 + 2, tci * 512 : (tci + 1) * 512]

            nc.vector.memset(scr1, 0.0)
            nc.vector.memset(wj.bitcast(mybir.dt.uint32), 0)
            nc.vector.memset(xj.bitcast(mybir.dt.uint32), 0)

            # prime the ACT Gelu table (no data deps: immediate scale/bias,
            # zeroed input). vd and all gouts ride the SP queue — anything on
            # the ACT HWDGE queue drains at ~25ns/descriptor and the kernel
            # epilogue spin-waits for that queue to empty (~9us measured).
            nc.scalar.activation(
                scr1, scr1, mybir.ActivationFunctionType.Gelu, bias=0.0, scale=0.0
            )
            # vd (2 KiB) rides the slow-but-idle ACT queue: lands ~10us,
            # first gelu needs it ~13us; frees the first SP slot for x/w
            nc.scalar.dma_start(vb1s, vd[:, :])

            # DMA schedule: bulk split across the two fast queues (the ACT
            # HWDGE queue is ~10x slower, bulk never goes there). x moves as
            # 32 per-(token-half u, k-chunk) DMAs of 128 KiB with 1024B
            # descriptors (~165 B/ns on SP); w as 4 quads with 2048B
            # descriptors. SWDGE is trigger-paced (~0.8us per DMA regardless
            # of size). The u0 half + w feed both tc0 and tc1 and are
            # emitted strictly in PE demand order, alternating queues.
            def w_dma(q, a):
                q.dma_start(ws[:, 4 * a : 4 * a + 4], w_slice(a))

            def w_pair_dma(q, j):
                # 128 KiB pair (k-chunks 2j, 2j+1) = column slice of the quad
                # rows: per-partition 1024B contiguous, host layout unchanged
                a, h = j // 2, j % 2
                q.dma_start(
                    ws[:, 4 * a + 2 * h : 4 * a + 2 * h + 2],
                    wd[a * P : (a + 1) * P, h * 1024 : (h + 1) * 1024].rearrange(
                        "p (jj r) -> p jj r", r=R
                    ),
                )

            def x_dma(q, u, k, half=None):
                base = (u * NK + k) * P
                if half is None:
                    q.dma_start(
                        xs[:, k, u * 1024 : (u + 1) * 1024],
                        xd[base : base + P, :],
                    )
                else:
                    lo = half * 512
                    q.dma_start(
                        xs[:, k, u * 1024 + lo : u * 1024 + lo + 512],
                        xd[base : base + P, lo : lo + 512],
                    )

            # emit w + u0 in PE demand order, alternating queues; u1 after
            # u0 head: each x pair lands BEFORE the w pair that serves it —
            # the w-first quad ordering cost a repeatable ~2us stall at the
            # k2=0->1 octet boundary (x2/x3 arriving ~1.5us late)
            items = []
            for j in range(NK2):
                items.append(("x", 2 * j))
                items.append(("x", 2 * j + 1))
                items.append(("w", j))
            for idx, (kind, val) in enumerate(items):
                q = nc.sync if idx % 2 == 0 else nc.gpsimd
                if kind == "w":
                    w_pair_dma(q, val)
                else:
                    x_dma(q, 0, val)
            for idx, k in enumerate(range(NK)):
                if k == NK - 1:
                    # last chunk (needed ~31us) on the slow ACT queue:
                    # lands ~20us even at 12 B/ns, frees a SWDGE trigger
                    x_dma(nc.scalar, 1, k)
                else:
                    x_dma(nc.sync if idx % 2 == 0 else nc.gpsimd, 1, k)

            def mm(banks, tci, rc, k2):
                nc.tensor.matmul(
                    banks[rc],
                    ws[:, 2 * k2 : 2 * k2 + 2, rc * P : (rc + 1) * P],
                    x_mov(tci, k2),
                    start=(k2 == 0), stop=(k2 == NK2 - 1),
                    perf_mode=mybir.MatmulPerfMode.DoubleRow,
                    skip_group_check=True,
                )

            def gelu(banks, tci, rc):
                g = gdump.tile([P, 512], F32, tag="g", name=f"g{tci}_{rc}")
                col = tci * NRC + rc
                nc.scalar.activation(
                    g, banks[rc], mybir.ActivationFunctionType.Gelu,
                    bias=vb1s[:, rc : rc + 1],
                    accum_out=gcol[:, col : col + 1],
                )

            def ship(tci):
                nc.sync.dma_start(
                    gout[:, tci * NRC : (tci + 1) * NRC],
                    gcol[:, tci * NRC : (tci + 1) * NRC],
                )

            banks0 = [
                psm.tile([P, 512], F32, tag="mm", name=f"mmA_{rc}")
                for rc in range(NRC)
            ]
            banks1 = [
                psm.tile([P, 512], F32, tag="mm", name=f"mmB_{rc}")
                for rc in range(NRC)
            ]

            # burn the PE DVFS p-state ramp on junk matmuls (results are
            # discarded; bank is reset by the first real start=True matmul).
            # Deliberately NOT bridged gaplessly into the real stream: a
            # fully gapless dense ignition measurably trips the chip power
            # throttle (2.4 -> 2.0 GHz for the whole run, +20% everywhere).
            for i in range(9):
                nc.tensor.matmul(
                    banks0[0], wj, xj, start=True, stop=True,
                    perf_mode=mybir.MatmulPerfMode.DoubleRow,
                    skip_group_check=True,
                )

            # tc0 and tc1 read the SAME u0 x chunks, so interleave them over
            # all 8 PSUM banks for the first k2 steps: per-chunk PE demand
            # halves exactly where DMA supply is leanest (no head stalls,
            # and the DVFS ramp overlaps the slack). Then tc0 finishes alone
            # and its gelus free banks 0-3 before tc2 needs them (don't
            # extend the interleave much further: that margin shrinks
            # ~0.9us per extra step and an ACT race eats the gain).
            ILV = 4
            for k2 in range(ILV):
                for banks, tci in ((banks0, 0), (banks1, 1)):
                    for rc in range(NRC):
                        mm(banks, tci, rc, k2)
            for k2 in range(ILV, NK2):
                for rc in range(NRC):
                    mm(banks0, 0, rc, k2)
            for rc in range(NRC):
                gelu(banks0, 0, rc)
            ship(0)
            for k2 in range(ILV, NK2):
                for rc in range(NRC):
                    mm(banks1, 1, rc, k2)
            for rc in range(NRC):
                gelu(banks1, 1, rc)
            ship(1)

            # tc2: k2-outer over 4 banks (reuses banks0's buffers)
            banks2 = [
                psm.tile([P, 512], F32, tag="mm", name=f"mmC_{rc}")
                for rc in range(NRC)
            ]
            for k2 in range(NK2):
                for rc in range(NRC):
                    mm(banks2, 2, rc, k2)
            for rc in range(NRC):
                gelu(banks2, 2, rc)
            ship(2)

            # tc3: rc-outer so the tail gelus overlap the last matmuls
            banks3 = [
                psm.tile([P, 512], F32, tag="mm", name=f"mmD_{rc}")
                for rc in range(NRC)
            ]
            for rc in range(NRC):
                for k2 in range(NK2):
                    mm(banks3, 3, rc, k2)
                gelu(banks3, 3, rc)
            ship(3)
    nc.finalize()
    return nc


def kernel(hidden_states, ln_gamma, ln_beta, w1, b1, w2, b2, wr, br):
    hs = np.asarray(hidden_states, dtype=np.float32)
    # LayerNorm on host in f32 (f64 accumulation for the stats)
    mu = hs.mean(-1, keepdims=True, dtype=np.float64)
    var = (hs.astype(np.float64) - mu).var(-1, keepdims=True)
    rstd = 1.0 / np.sqrt(var + LN_EPS)
    xn8 = ((hs - mu.astype(np.float32)) * rstd.astype(np.float32)).astype(
        ml_dtypes.float8_e4m3fn
    )

    g64 = np.asarray(ln_gamma, dtype=np.float64)
    be64 = np.asarray(ln_beta, dtype=np.float64)
    w1_64 = np.asarray(w1, dtype=np.float64)
    w1g8 = (g64[:, None] * w1_64).astype(np.float32).astype(ml_dtypes.float8_e4m3fn)
    vb1 = (be64 @ w1_64 + np.asarray(b1, np.float64)).astype(np.float32)

    # device layouts (see _build):
    #   wd[(a*128+p), (jj*512+r)] = w1g8[(4*a+jj)*128+p, r]
    wdh = np.ascontiguousarray(
        w1g8.reshape(4, 4, P, R).transpose(0, 2, 1, 3).reshape(4 * P, 4 * R)
    )
    vdh = np.ascontiguousarray(vb1.reshape(NRC, P).T)  # [128, 4]

    if "nc" not in _cache:
        _cache["nc"] = _build()
    nc = _cache["nc"]

    in_maps = []
    for b in range(N_CORES):
        #   xd[((u*16+k)*128+p), t] = xn8[b, u*1024+t, k*128+p]
        xT = np.ascontiguousarray(xn8[b].T)  # [D, S]
        xdh = np.ascontiguousarray(
            xT.reshape(NK, P, 2, 1024)
            .transpose(2, 0, 1, 3)
            .reshape(2 * NK * P, 1024)
        )
        in_maps.append({"xd": xdh, "wd": wdh, "vd": vdh})
    res = run_bass_kernel_spmd(nc, in_maps, core_ids=list(range(N_CORES)))
    gaccs = np.stack([res.results[b]["gout"] for b in range(N_CORES)], axis=0)
    global _last_res
    _last_res = res

    # host tail in fp64 (tiny): H -> w2 -> router -> aux/next_idx
    # gcol[p, tc*4+rc] = sum over token chunk tc of gelu row r = rc*128+p
    H = (
        gaccs.astype(np.float64)
        .reshape(B, P, NTC, NRC)
        .sum(axis=2)            # [B, p, rc]
        .transpose(0, 2, 1)     # [B, rc, p]
        .reshape(B, R)
    )
    bt = H @ np.asarray(w2, np.float64) + float(S) * np.asarray(b2, np.float64)
    logits = bt @ np.asarray(wr, np.float64) + np.asarray(br, np.float64)  # [B, E]
    global _last_logits
    _last_logits = logits.astype(np.float32)

    idx = logits.argmax(axis=-1)
    targets = np.zeros_like(logits)
    targets[np.arange(B), idx] = 1.0
    aux = (np.logaddexp(0.0, logits) - logits * targets).mean()
    counts = targets.sum(0)
    next_idx = int(np.argmax(counts))
    return np.float32(aux), np.int32(next_idx)
